# revision 39
# baseline (speedup 1.0000x reference)
"""MultiHeadAttention kernel for Trainium2 (8 NeuronCores, data-parallel over batch).

Reference computation (B=8, S=2048, D=64, concat=768):
    q = x @ Wq.T ; k = x @ Wk.T ; v = x @ Wv.T          # [B,S,768]
    scores = (q @ k.T) / sqrt(64)                        # [B,S,S]  (full concat dim!)
    attn = softmax(scores, -1)
    out = (attn @ v) @ Wf.T + b                          # [B,S,64]

Key algebraic identity: since the scores contract over the FULL concat dim,
q @ k.T = x (Wq^T Wk) x^T with A := Wq^T Wk in R^{64x64}; similarly
(attn @ v) @ Wf^T = attn @ (x @ W2) with W2 := Wv^T Wf^T in R^{64x64}.
A and W2 are weight-only folds, precomputed on the host at load time; all
activation-dependent math (y = A^T x^T, z = x W2 + b, scores, softmax, O')
runs on device. The softmax denominator comes from a ones column appended to
z (bias folded into z, so the final division yields attn@z + b directly).

Layout: scores are computed TRANSPOSED ([key chunk = 128 partitions, query
free]) so the exp output feeds the O' matmul directly. Score matmuls for two
key chunks are row-packed into PE row groups 0-63 / 64-127 (concurrent on
the array). Finalize copies O'^T+denominator to SBUF in one full-width op,
PE-transposes it back to query-major, then reciprocal+multiply; the output
DRAM layout is [j, p, q, d] so every DMA line is 1KB contiguous (the host
un-permutes — pure marshaling).

Scheduling: O' matmuls and finalize transposes trail through FIFOs popped
once per slot, so PE work rides between score matmuls, the PSUM accumulator
reuse across query halves never stalls, and the HAM clock gate (which only
un-throttles the PE to 2.4 GHz after a ~3.4us fully-busy window) engages
early — dep-free warmup fillers cover the DMA-wait and prep dep bubbles.

Precision: score matmuls are bf16; exp outputs and z are fp8e4 so the O'
accumulation runs in DoubleRow mode. The exp is split across ACT (exact) and
Vector (Schraudolph fast exp: round(s*8*log2e + B) written as uint8 IS the
fp8e4 bit pattern of exp(s/8)), balanced by a static greedy over measured
per-block engine costs. PSUM accumulation and the normalize path stay fp32.
"""

import sys

sys.path.insert(0, "/opt/trn_rl_repo")

from collections import deque

import ml_dtypes
import numpy as np

import concourse.bass as bass
import concourse.tile as tile
from concourse import bacc, mybir
from concourse.bass_utils import run_bass_kernel_spmd

F32 = mybir.dt.float32
F32R = mybir.dt.float32r
BF16 = mybir.dt.bfloat16
FP8 = mybir.dt.float8e4
U8 = mybir.dt.uint8
ALU = mybir.AluOpType
DRM = mybir.MatmulPerfMode.DoubleRow

B, S, D, C = 8, 2048, 64, 768
NCHUNK = S // 128          # 16 key chunks of 128
NPAIR = NCHUNK // 2        # 8 row-packed chunk pairs
NSUP = S // 512            # 4 query superblocks of 512
SCALING = 0.125            # 1/sqrt(64)
ZP = 80                    # z row pitch (DoubleRow needs step % 16 == 0)
OLAG = 3                   # O' matmuls trail their scores by this many slots
# Schraudolph fast-exp constants (uint8 result IS the fp8e4 bit pattern);
# round-to-nearest convert measured on HW, c=0.35 tuned for it
SCH_A = float(SCALING * 8 * np.log2(np.e))
SCH_B = float(56.0 - 0.35)
# measured per-[128x1024]-block exp cost (us) used by the greedy balancer
COST_ACT, COST_VEC = 1.12, 1.22
COST_COPY = 0.57           # [*,512] fp32 psum->sbuf copy, either engine


def _build_nc():
    nc = bacc.Bacc("TRN2", target_bir_lowering=False, debug=False)

    a_d = nc.dram_tensor("a", [D, D], BF16, kind="ExternalInput")
    w2_d = nc.dram_tensor("w2", [128, D], BF16, kind="ExternalInput")
    xT_d = nc.dram_tensor("xT", [128, S], BF16, kind="ExternalInput")
    b_d = nc.dram_tensor("b_final", [D], F32, kind="ExternalInput")
    ident_d = nc.dram_tensor("ident", [128, 128], F32R, kind="ExternalInput")
    out_d = nc.dram_tensor("out", [NSUP, 128, 4, D], F32, kind="ExternalOutput")

    with tile.TileContext(nc) as tc:
        _emit(tc, a_d, w2_d, xT_d, b_d, ident_d, out_d)
    nc.compile()
    return nc


def _emit(tc, a_d, w2_d, xT_d, b_d, ident_d, out_d):
    nc = tc.nc
    const = tc.alloc_tile_pool(name="const", bufs=1)

    # dep-free first PE instruction: triggers the PE IRAM instruction fetch
    # at t=0 instead of after the first operand DMA lands
    nc.tensor.nop(nofuse=True)

    # ---- input DMAs, spread across queues so the x^T halves land in parallel
    wtile = const.tile([128, 512], BF16)
    nc.vector.memset(wtile[:], 0.5)
    a_sb = const.tile([D, D], BF16)
    nc.scalar.dma_start(a_sb[:], a_d.ap())
    xTd = const.tile([128, S], BF16)
    xT_ap = xT_d.ap()
    nc.sync.dma_start(xTd[:, 0:1024], xT_ap[:, 0:1024])
    nc.scalar.dma_start(xTd[:, 1024:2048], xT_ap[:, 1024:2048])
    ident = const.tile([128, 128], F32R)
    nc.sync.dma_start(ident[:], ident_d.ap())

    # ---- gpsimd (SWDGE) queue: W2 (dup'd on host), bias, z constants
    w2_sb = const.tile([128, D], BF16)
    nc.gpsimd.dma_start(w2_sb[:], w2_d.ap())
    b_bcast = const.tile([128, D], F32)
    b_ap = b_d.ap()
    b_src = bass.AP(tensor=b_ap.tensor, offset=b_ap.offset, ap=[[0, 128]] + list(b_ap.ap))
    nc.gpsimd.dma_start(b_bcast[:], b_src)

    # z (DoubleRow stationary): cols 0-63 = x@W2+b, col 64 = ones (softmax
    # denominator), cols 65-79 = zero pad for the 16-aligned pitch
    z_sb = const.tile([128, NCHUNK, ZP], FP8)
    nc.gpsimd.memset(z_sb[:, :, D : D + 1], 1.0)
    nc.gpsimd.memset(z_sb[:, :, D + 1 : ZP], 0)

    # warm the ACT exp table early so the table load overlaps the DMA phase
    warm = const.tile([1, 1], F32)
    nc.scalar.activation(out=warm[:], in_=wtile[0:1, 0:2].bitcast(F32),
                         func=mybir.ActivationFunctionType.Exp, scale=1.0)

    yTd = const.tile([128, S], BF16)       # y^T = A^T x^T, rows 0-63 and 64-127

    # PSUM pool lifetimes chain (prep -> main) without nesting: prep uses
    # 2-deep score rotation (4 banks) + 2 scratch + 1 warmup bank; after
    # prep releases, main gets a 3-deep score rotation (6) + 2 accumulators.
    scp_prep = tc.alloc_tile_pool(name="sc_ps_p", bufs=2, space="PSUM")
    etp = tc.alloc_tile_pool(name="et", bufs=4)
    fsb = tc.alloc_tile_pool(name="fin_sb", bufs=2)
    osb = tc.alloc_tile_pool(name="out_sb", bufs=2)
    pps = tc.alloc_tile_pool(name="prep_ps", bufs=1, space="PSUM")

    # dep-free warmup/filler matmuls: keep the PE busy through the HAM
    # SHORT window and across prep's cross-engine dep bubbles so the clock
    # gate reaches (and keeps) K=8/8. Interleaved between real prep work.
    pe_warm = pps.tile([128, 512], F32, tag="warm", bufs=1, name="pe_warm")

    def pe_filler(n=512):
        nc.tensor.matmul(pe_warm[:, 0:n], wtile[:, 0:128], wtile[:, 0:n],
                         start=True, stop=True)

    # dense spin through a full HAM SHORT window so the clock gate opens
    # before the real stream; the per-j fillers below then keep it open
    for _ in range(14):
        pe_filler(256)

    # ---- greedy ACT/Vector balance for exp blocks and psum->sbuf copies
    ebusy = {"act": 0.3, "vec": 0.3}

    def pick(cost_act, cost_vec):
        e = "act" if ebusy["act"] + cost_act <= ebusy["vec"] + cost_vec else "vec"
        ebusy[e] += cost_act if e == "act" else cost_vec
        return e

    def bal_copy(out, in_):
        if pick(COST_COPY, COST_COPY + 0.11) == "vec":
            nc.vector.tensor_copy(out, in_)
        else:
            nc.scalar.copy(out, in_)

    def scores_exp_j(p, jg, jl, eT, pool, eng=None):
        # jg = global query superblock (0-3), jl = slot within the half (0/1)
        n0, n1 = 2 * p, 2 * p + 1
        sc = pool.tile([128, 1024], F32, tag="sc", name=f"sc{p}_{jg}")
        nc.tensor.matmul(sc[:, 0:512], xTd[0:D, n0 * 128 : (n0 + 1) * 128],
                         yTd[0:D, jg * 512 : (jg + 1) * 512],
                         start=True, stop=True)
        nc.tensor.matmul(sc[:, 512:1024], xTd[D:128, n1 * 128 : (n1 + 1) * 128],
                         yTd[D:128, jg * 512 : (jg + 1) * 512],
                         start=True, stop=True)
        if eng is not None:
            ebusy[eng] += COST_ACT if eng == "act" else COST_VEC
        else:
            eng = pick(COST_ACT, COST_VEC)
        if eng == "vec":
            # Schraudolph fast exp on Vector: round(s*A + B) as uint8 bits
            nc.vector.tensor_scalar(eT[:, jl, :, :].bitcast(U8), sc[:],
                                    SCH_A, SCH_B, ALU.mult, ALU.add)
        else:
            nc.scalar.activation(out=eT[:, jl, :, :], in_=sc[:],
                                 func=mybir.ActivationFunctionType.Exp,
                                 scale=SCALING)

    def new_eT(h, p):
        # [keys 128][half-local j 2][chunk 2][512]
        return etp.tile([128, 2, 2, 512], FP8, tag="et", bufs=4, name=f"eT{h}_{p}")

    # prep: per j-block, the two y^T matmuls (both partition halves via
    # tile_position), one full-width copy, the half-0 pair-0 scores, and the
    # first z pair. Dep-free fillers cover the cross-engine bubbles.
    eT00 = new_eT(0, 0)
    for j in range(NSUP):
        yp = pps.tile([128, 512], F32, tag="t2", bufs=2, name=f"yp{j}")
        nc.tensor.matmul(yp[0:D, :], a_sb[:], xTd[0:D, j * 512 : (j + 1) * 512],
                         start=True, stop=True)
        nc.tensor.matmul(yp[D:128, :], a_sb[:], xTd[0:D, j * 512 : (j + 1) * 512],
                         start=True, stop=True)
        pe_filler()
        # explicit prep assignment: keep the copy off the engine that owns
        # this j-block's exp so the chains pipeline instead of serializing
        if j % 2 == 0:
            nc.vector.tensor_copy(yTd[:, j * 512 : (j + 1) * 512], yp[:, :])
            ebusy["vec"] += 0.68
        else:
            nc.scalar.copy(yTd[:, j * 512 : (j + 1) * 512], yp[:, :])
            ebusy["act"] += COST_COPY
        if j < 2:
            scores_exp_j(0, j, j, eT00, scp_prep,
                         eng="act" if j == 0 else "vec")
            pe_filler()

    # z pair 0 (chunks 0/1) must beat O'(p=0) popping early in main
    zp0 = pps.tile([128, 512], F32, tag="t2", bufs=2, name="zp0")
    zp1 = pps.tile([128, 512], F32, tag="t2", bufs=2, name="zp1")
    nc.tensor.matmul(zp0[:, 0:D], xTd[0:D, 0:128], w2_sb[0:D, :],
                     start=True, stop=True)
    nc.tensor.matmul(zp1[:, 0:D], xTd[D:128, 128:256], w2_sb[D:128, :],
                     start=True, stop=True)
    pe_filler()
    nc.vector.tensor_add(z_sb[:, 0, 0:D], zp0[:, 0:D], b_bcast[:])
    nc.vector.tensor_add(z_sb[:, 1, 0:D], zp1[:, 0:D], b_bcast[:])
    ebusy["vec"] += 0.52

    pps.release()
    scp_prep.release()
    scp = tc.alloc_tile_pool(name="sc_ps", bufs=3, space="PSUM")

    # ---- main loop over two query halves. O' matmuls and finalize
    # transposes pop from FIFOs at one per slot, riding between score
    # matmuls; the half-boundary PSUM accumulator reuse (h1's p=0 start=True
    # write into h0's banks) happens OLAG slots after h0's finalize copy.
    oacc_pool = tc.alloc_tile_pool(name="oacc", bufs=1, space="PSUM")
    out_ap = out_d.ap()

    def oprime_j(p, jl, eT, o_ps):
        nc.tensor.matmul(o_ps[jl][:], z_sb[:, 2 * p : 2 * p + 2, :],
                         eT[:, jl, :, :],
                         start=(p == 0), stop=(p == NPAIR - 1),
                         perf_mode=DRM)

    def z_pair_main(zh):
        # deferred z pairs 1-7, two bank-aligned outputs in one sc slot
        n0, n1 = 2 * zh, 2 * zh + 1
        zp = scp.tile([128, 1024], F32, tag="sc", name=f"zpd{zh}")
        nc.tensor.matmul(zp[:, 0:D], xTd[0:D, n0 * 128 : (n0 + 1) * 128],
                         w2_sb[0:D, :], start=True, stop=True)
        nc.tensor.matmul(zp[:, 512 : 512 + D], xTd[D:128, n1 * 128 : (n1 + 1) * 128],
                         w2_sb[D:128, :], start=True, stop=True)
        nc.vector.tensor_add(z_sb[:, n0, 0:D], zp[:, 0:D], b_bcast[:])
        nc.vector.tensor_add(z_sb[:, n1, 0:D], zp[:, 512 : 512 + D], b_bcast[:])
        ebusy["vec"] += 0.52

    # finalize: one full-width psum->sbuf copy (rows 0-65: data, denom, zero
    # pad straight from the accumulator), 4 PE transposes (interleaved one
    # per slot), then reciprocal+multiply and a 1KB-line output DMA
    ot_tiles = [const.tile([D + 2, 512], F32R, name=f"ot{i}") for i in range(2)]

    def finalize_copy(h, jl, o_ps):
        ot = ot_tiles[jl]
        bal_copy(ot[:, :], o_ps[jl][0 : D + 2, :])
        pt = scp.tile([128, 4, D + 2], F32R, tag="sc", name=f"fin{2*h+jl}")
        return ot, pt

    def finalize_end(h, jl, pt):
        jg = 2 * h + jl
        r_sb = fsb.tile([128, 4], F32, tag="r")
        nc.vector.reciprocal(r_sb[:], pt[:, :, D : D + 1].bitcast(F32))
        o_out = osb.tile([128, 4, D], F32, tag="oo")
        nc.vector.tensor_mul(o_out[:], pt[:, :, 0:D],
                             r_sb[:].unsqueeze(2).broadcast_to([128, 4, D]))
        nc.sync.dma_start(out_ap[jg], o_out[:])
        ebusy["vec"] += 0.71

    ofifo = deque()        # (h, p, jl, eT, o_ps) awaiting O' emission
    tfifo = deque()        # pending finalize transposes, one PE op each

    def pop_oprime():
        hh, pp, jll, eTT, po = ofifo.popleft()
        oprime_j(pp, jll, eTT, po)
        if pp == NPAIR - 1:
            ot, pt = finalize_copy(hh, jll, po)
            for q in range(4):
                tfifo.append((hh, jll, ot, pt, q))

    def pop_transpose():
        hh, jll, ot, pt, q = tfifo.popleft()
        nc.tensor.transpose(pt[:, q, :], ot[:, q * 128 : (q + 1) * 128],
                            ident[0 : D + 2, 0 : D + 2])
        if q == 3:
            finalize_end(hh, jll, pt)

    for h in range(2):
        o_ps = [oacc_pool.tile([ZP, 512], F32, tag=f"o{jl}", name=f"o_ps{h}_{jl}")
                for jl in range(2)]
        for p in range(NPAIR):
            if h == 0 and p == 0:
                for jl in range(2):
                    ofifo.append((0, 0, jl, eT00, o_ps))
                continue
            if h == 0:
                z_pair_main(p)
            if h == 0 and p == 1:
                # dep-free fillers into the still-unwritten O' accumulator
                # banks (their start=True write comes slots later): PE
                # density for the HAM warmup through the main-loop ramp
                for jl in range(2):
                    nc.tensor.matmul(o_ps[jl][:, :], wtile[:, 0:ZP],
                                     wtile[:], start=True, stop=True)
            eT = new_eT(h, p)
            for jl in range(2):
                scores_exp_j(p, 2 * h + jl, jl, eT, scp)
                ofifo.append((h, p, jl, eT, o_ps))
                # steady-state lag OLAG; collapse gradually over the last
                # two pairs of the last half so the tail is short
                if h == 1 and p >= NPAIR - 2:
                    target = 2 * (NPAIR - 1 - p) + (1 - jl)
                else:
                    target = OLAG
                while len(ofifo) > target:
                    pop_oprime()
                if tfifo:
                    pop_transpose()
                    if h == 1 and p >= NPAIR - 2 and tfifo:
                        pop_transpose()

    # tail: drain the FIFOs
    while ofifo:
        pop_oprime()
        if tfifo:
            pop_transpose()
    while tfifo:
        pop_transpose()

    osb.release()
    fsb.release()
    oacc_pool.release()
    etp.release()
    scp.release()
    const.release()


_NC_CACHE = {}


def _get_nc():
    if "nc" not in _NC_CACHE:
        _NC_CACHE["nc"] = _build_nc()
    return _NC_CACHE["nc"]


def kernel(x, w_q, w_k, w_v, w_final, b_final, _trace=False):
    nc = _get_nc()
    bfr = lambda a: np.asarray(a, dtype=np.float32).astype(ml_dtypes.bfloat16).astype(np.float32)
    # weight-only folds (load-time preprocessing): A = Wq^T Wk, W2 = Wv^T Wf^T
    A = (bfr(w_q).T @ bfr(w_k)).astype(ml_dtypes.bfloat16)
    W2 = (bfr(w_v).T @ bfr(w_final).T).astype(ml_dtypes.bfloat16)
    W2d = np.ascontiguousarray(np.concatenate([W2, W2], axis=0))  # both halves
    xb = np.asarray(x, dtype=np.float32).astype(ml_dtypes.bfloat16)  # [B,S,D]
    # host-side layout marshaling: x^T per batch, duplicated onto both
    # partition halves for the row-packed score matmuls
    xT = np.ascontiguousarray(np.concatenate([xb.transpose(0, 2, 1),
                                              xb.transpose(0, 2, 1)], axis=1))
    shared = {
        "a": np.ascontiguousarray(A),
        "w2": W2d,
        "b_final": np.ascontiguousarray(np.asarray(b_final, dtype=np.float32)),
        "ident": np.eye(128, dtype=np.float32),
    }
    in_maps = [dict(shared, xT=xT[b]) for b in range(B)]
    res = run_bass_kernel_spmd(nc, in_maps, core_ids=list(range(B)), trace=_trace)
    # device output is [j, p, q, d]; un-permute to [S, 64] (layout only)
    out = np.stack([res.results[b]["out"] for b in range(B)], axis=0)
    out = np.ascontiguousarray(
        out.transpose(0, 1, 3, 2, 4).reshape(B, S, D))
    if _trace:
        return out, res
    return out


# revision 40
# speedup vs baseline: 1.1424x; 1.1424x over previous
"""MultiHeadAttention kernel for Trainium2 (8 NeuronCores, data-parallel over batch).

Reference computation (B=8, S=2048, D=64, concat=768):
    q = x @ Wq.T ; k = x @ Wk.T ; v = x @ Wv.T          # [B,S,768]
    scores = (q @ k.T) / sqrt(64)                        # [B,S,S]  (full concat dim!)
    attn = softmax(scores, -1)
    out = (attn @ v) @ Wf.T + b                          # [B,S,64]

Key algebraic identity: since the scores contract over the FULL concat dim,
q @ k.T = x (Wq^T Wk) x^T with A := Wq^T Wk in R^{64x64}; similarly
(attn @ v) @ Wf^T = attn @ (x @ W2) with W2 := Wv^T Wf^T in R^{64x64}.
A and W2 are weight-only folds, precomputed on the host at load time; all
activation-dependent math (y = A^T x^T, z = x W2 + b, scores, softmax, O')
runs on device. The softmax denominator comes from a ones column appended to
z (bias folded into z, so the final division yields attn@z + b directly).

Layout: scores are computed TRANSPOSED ([key chunk = 128 partitions, query
free]) so the exp output feeds the O' matmul directly. Score matmuls for two
key chunks are row-packed into PE row groups 0-63 / 64-127 (concurrent on
the array). Finalize copies O'^T+denominator to SBUF in one full-width op,
PE-transposes it back to query-major, then reciprocal+multiply; the output
DRAM layout is [j, p, q, d] so every DMA line is 1KB contiguous (the host
un-permutes — pure marshaling).

Scheduling: O' matmuls and finalize transposes trail through FIFOs popped
once per slot, so PE work rides between score matmuls, the PSUM accumulator
reuse across query halves never stalls, and the HAM clock gate (which only
un-throttles the PE to 2.4 GHz after a ~3.4us fully-busy window) engages
early — dep-free warmup fillers cover the DMA-wait and prep dep bubbles.

Precision: score matmuls are bf16; exp outputs and z are fp8e4 so the O'
accumulation runs in DoubleRow mode. The exp is split across ACT (exact) and
Vector (Schraudolph fast exp: round(s*8*log2e + B) written as uint8 IS the
fp8e4 bit pattern of exp(s/8)), balanced by a static greedy over measured
per-block engine costs. PSUM accumulation and the normalize path stay fp32.
"""

import sys

sys.path.insert(0, "/opt/trn_rl_repo")

from collections import deque

import ml_dtypes
import numpy as np

import concourse.bass as bass
import concourse.tile as tile
from concourse import bacc, mybir
from concourse.bass_utils import run_bass_kernel_spmd

F32 = mybir.dt.float32
F32R = mybir.dt.float32r
BF16 = mybir.dt.bfloat16
FP8 = mybir.dt.float8e4
U8 = mybir.dt.uint8
ALU = mybir.AluOpType
DRM = mybir.MatmulPerfMode.DoubleRow

B, S, D, C = 8, 2048, 64, 768
NCHUNK = S // 128          # 16 key chunks of 128
NPAIR = NCHUNK // 2        # 8 row-packed chunk pairs
NSUP = S // 512            # 4 query superblocks of 512
SCALING = 0.125            # 1/sqrt(64)
ZP = 80                    # z row pitch (DoubleRow needs step % 16 == 0)
OLAG = 3                   # O' matmuls trail their scores by this many slots
# Schraudolph fast-exp constants (uint8 result IS the fp8e4 bit pattern);
# round-to-nearest convert measured on HW, c=0.35 tuned for it
SCH_A = float(SCALING * 8 * np.log2(np.e))
SCH_B = float(56.0 - 0.35)
# measured per-[128x1024]-block exp cost (us) used by the greedy balancer
COST_ACT, COST_VEC = 1.12, 1.22
COST_COPY = 0.57           # [*,512] fp32 psum->sbuf copy, either engine


def _build_nc():
    nc = bacc.Bacc("TRN2", target_bir_lowering=False, debug=False)

    a_d = nc.dram_tensor("a", [D, D], BF16, kind="ExternalInput")
    w2_d = nc.dram_tensor("w2", [128, D], BF16, kind="ExternalInput")
    xT_d = nc.dram_tensor("xT", [128, S], BF16, kind="ExternalInput")
    b_d = nc.dram_tensor("b_final", [D], F32, kind="ExternalInput")
    ident_d = nc.dram_tensor("ident", [128, 128], F32R, kind="ExternalInput")
    out_d = nc.dram_tensor("out", [NSUP, 128, 4, D], F32, kind="ExternalOutput")

    with tile.TileContext(nc) as tc:
        _emit(tc, a_d, w2_d, xT_d, b_d, ident_d, out_d)
    nc.compile()
    return nc


def _emit(tc, a_d, w2_d, xT_d, b_d, ident_d, out_d):
    nc = tc.nc
    const = tc.alloc_tile_pool(name="const", bufs=1)

    # dep-free first PE instruction: triggers the PE IRAM instruction fetch
    # at t=0 instead of after the first operand DMA lands
    nc.tensor.nop(nofuse=True)

    # ---- input DMAs, spread across queues so the x^T halves land in parallel
    wtile = const.tile([128, 512], BF16)
    nc.vector.memset(wtile[:], 0.5)
    a_sb = const.tile([D, D], BF16)
    nc.scalar.dma_start(a_sb[:], a_d.ap())
    xTd = const.tile([128, S], BF16)
    xT_ap = xT_d.ap()
    nc.sync.dma_start(xTd[:, 0:1024], xT_ap[:, 0:1024])
    nc.scalar.dma_start(xTd[:, 1024:2048], xT_ap[:, 1024:2048])
    ident = const.tile([128, 128], F32R)
    nc.sync.dma_start(ident[:], ident_d.ap())

    # ---- gpsimd (SWDGE) queue: W2 (dup'd on host), bias, z constants
    w2_sb = const.tile([128, D], BF16)
    nc.gpsimd.dma_start(w2_sb[:], w2_d.ap())
    b_bcast = const.tile([128, D], F32)
    b_ap = b_d.ap()
    b_src = bass.AP(tensor=b_ap.tensor, offset=b_ap.offset, ap=[[0, 128]] + list(b_ap.ap))
    nc.gpsimd.dma_start(b_bcast[:], b_src)

    # z (DoubleRow stationary): cols 0-63 = x@W2+b, col 64 = ones (softmax
    # denominator), cols 65-79 = zero pad for the 16-aligned pitch
    z_sb = const.tile([128, NCHUNK, ZP], FP8)
    nc.gpsimd.memset(z_sb[:, :, D : D + 1], 1.0)
    nc.gpsimd.memset(z_sb[:, :, D + 1 : ZP], 0)

    # warm the ACT exp table early so the table load overlaps the DMA phase
    warm = const.tile([1, 1], F32)
    nc.scalar.activation(out=warm[:], in_=wtile[0:1, 0:2].bitcast(F32),
                         func=mybir.ActivationFunctionType.Exp, scale=1.0)

    yTd = const.tile([128, S], BF16)       # y^T = A^T x^T, rows 0-63 and 64-127

    # PSUM pool lifetimes chain (prep -> main) without nesting: prep uses
    # 2-deep score rotation (4 banks) + 2 scratch + 1 warmup bank; after
    # prep releases, main gets a 3-deep score rotation (6) + 2 accumulators.
    scp_prep = tc.alloc_tile_pool(name="sc_ps_p", bufs=2, space="PSUM")
    etp = tc.alloc_tile_pool(name="et", bufs=4)
    fsb = tc.alloc_tile_pool(name="fin_sb", bufs=2)
    osb = tc.alloc_tile_pool(name="out_sb", bufs=2)
    pps = tc.alloc_tile_pool(name="prep_ps", bufs=1, space="PSUM")

    # dep-free warmup/filler matmuls: keep the PE busy through the HAM
    # SHORT window and across prep's cross-engine dep bubbles so the clock
    # gate reaches (and keeps) K=8/8. Interleaved between real prep work.
    pe_warm = pps.tile([128, 512], F32, tag="warm", bufs=1, name="pe_warm")

    def pe_filler(n=512):
        nc.tensor.matmul(pe_warm[:, 0:n], wtile[:, 0:128], wtile[:, 0:n],
                         start=True, stop=True)

    for _ in range(3):
        pe_filler()

    # ---- greedy ACT/Vector balance for exp blocks and psum->sbuf copies
    ebusy = {"act": 0.3, "vec": 0.3}

    def pick(cost_act, cost_vec):
        e = "act" if ebusy["act"] + cost_act <= ebusy["vec"] + cost_vec else "vec"
        ebusy[e] += cost_act if e == "act" else cost_vec
        return e

    def bal_copy(out, in_):
        if pick(COST_COPY, COST_COPY + 0.11) == "vec":
            nc.vector.tensor_copy(out, in_)
        else:
            nc.scalar.copy(out, in_)

    def scores_exp_j(p, jg, jl, eT, pool, eng=None):
        # jg = global query superblock (0-3), jl = slot within the half (0/1)
        n0, n1 = 2 * p, 2 * p + 1
        sc = pool.tile([128, 1024], F32, tag="sc", name=f"sc{p}_{jg}")
        nc.tensor.matmul(sc[:, 0:512], xTd[0:D, n0 * 128 : (n0 + 1) * 128],
                         yTd[0:D, jg * 512 : (jg + 1) * 512],
                         start=True, stop=True)
        nc.tensor.matmul(sc[:, 512:1024], xTd[D:128, n1 * 128 : (n1 + 1) * 128],
                         yTd[D:128, jg * 512 : (jg + 1) * 512],
                         start=True, stop=True)
        if eng is not None:
            ebusy[eng] += COST_ACT if eng == "act" else COST_VEC
        else:
            eng = pick(COST_ACT, COST_VEC)
        if eng == "vec":
            # Schraudolph fast exp on Vector: round(s*A + B) as uint8 bits
            nc.vector.tensor_scalar(eT[:, jl, :, :].bitcast(U8), sc[:],
                                    SCH_A, SCH_B, ALU.mult, ALU.add)
        else:
            nc.scalar.activation(out=eT[:, jl, :, :], in_=sc[:],
                                 func=mybir.ActivationFunctionType.Exp,
                                 scale=SCALING)

    def new_eT(h, p):
        # [keys 128][half-local j 2][chunk 2][512]
        return etp.tile([128, 2, 2, 512], FP8, tag="et", bufs=4, name=f"eT{h}_{p}")

    # prep: per j-block, the two y^T matmuls (both partition halves via
    # tile_position), one full-width copy, the half-0 pair-0 scores, and the
    # first z pair. Dep-free fillers cover the cross-engine bubbles.
    eT00 = new_eT(0, 0)
    for j in range(NSUP):
        yp = pps.tile([128, 512], F32, tag="t2", bufs=2, name=f"yp{j}")
        nc.tensor.matmul(yp[0:D, :], a_sb[:], xTd[0:D, j * 512 : (j + 1) * 512],
                         start=True, stop=True)
        nc.tensor.matmul(yp[D:128, :], a_sb[:], xTd[0:D, j * 512 : (j + 1) * 512],
                         start=True, stop=True)
        pe_filler()
        # explicit prep assignment: keep the copy off the engine that owns
        # this j-block's exp so the chains pipeline instead of serializing
        if j % 2 == 0:
            nc.vector.tensor_copy(yTd[:, j * 512 : (j + 1) * 512], yp[:, :])
            ebusy["vec"] += 0.68
        else:
            nc.scalar.copy(yTd[:, j * 512 : (j + 1) * 512], yp[:, :])
            ebusy["act"] += COST_COPY
        if j < 2:
            scores_exp_j(0, j, j, eT00, scp_prep,
                         eng="act" if j == 0 else "vec")
            pe_filler()

    # z pair 0 (chunks 0/1) must beat O'(p=0) popping early in main
    zp0 = pps.tile([128, 512], F32, tag="t2", bufs=2, name="zp0")
    zp1 = pps.tile([128, 512], F32, tag="t2", bufs=2, name="zp1")
    nc.tensor.matmul(zp0[:, 0:D], xTd[0:D, 0:128], w2_sb[0:D, :],
                     start=True, stop=True)
    nc.tensor.matmul(zp1[:, 0:D], xTd[D:128, 128:256], w2_sb[D:128, :],
                     start=True, stop=True)
    pe_filler()
    nc.vector.tensor_add(z_sb[:, 0, 0:D], zp0[:, 0:D], b_bcast[:])
    nc.vector.tensor_add(z_sb[:, 1, 0:D], zp1[:, 0:D], b_bcast[:])
    ebusy["vec"] += 0.52

    pps.release()
    scp_prep.release()
    scp = tc.alloc_tile_pool(name="sc_ps", bufs=3, space="PSUM")

    # ---- main loop over two query halves. O' matmuls and finalize
    # transposes pop from FIFOs at one per slot, riding between score
    # matmuls; the half-boundary PSUM accumulator reuse (h1's p=0 start=True
    # write into h0's banks) happens OLAG slots after h0's finalize copy.
    oacc_pool = tc.alloc_tile_pool(name="oacc", bufs=1, space="PSUM")
    out_ap = out_d.ap()

    def oprime_j(p, jl, eT, o_ps):
        nc.tensor.matmul(o_ps[jl][:], z_sb[:, 2 * p : 2 * p + 2, :],
                         eT[:, jl, :, :],
                         start=(p == 0), stop=(p == NPAIR - 1),
                         perf_mode=DRM)

    def z_pair_main(zh):
        # deferred z pairs 1-7, two bank-aligned outputs in one sc slot
        n0, n1 = 2 * zh, 2 * zh + 1
        zp = scp.tile([128, 1024], F32, tag="sc", name=f"zpd{zh}")
        nc.tensor.matmul(zp[:, 0:D], xTd[0:D, n0 * 128 : (n0 + 1) * 128],
                         w2_sb[0:D, :], start=True, stop=True)
        nc.tensor.matmul(zp[:, 512 : 512 + D], xTd[D:128, n1 * 128 : (n1 + 1) * 128],
                         w2_sb[D:128, :], start=True, stop=True)
        nc.vector.tensor_add(z_sb[:, n0, 0:D], zp[:, 0:D], b_bcast[:])
        nc.vector.tensor_add(z_sb[:, n1, 0:D], zp[:, 512 : 512 + D], b_bcast[:])
        ebusy["vec"] += 0.52

    # finalize: one full-width psum->sbuf copy (rows 0-65: data, denom, zero
    # pad straight from the accumulator), 4 PE transposes (interleaved one
    # per slot), then reciprocal+multiply and a 1KB-line output DMA
    ot_tiles = [const.tile([D + 2, 512], F32R, name=f"ot{i}") for i in range(2)]

    def finalize_copy(h, jl, o_ps):
        ot = ot_tiles[jl]
        bal_copy(ot[:, :], o_ps[jl][0 : D + 2, :])
        pt = scp.tile([128, 4, D + 2], F32R, tag="sc", name=f"fin{2*h+jl}")
        return ot, pt

    def finalize_end(h, jl, pt):
        jg = 2 * h + jl
        r_sb = fsb.tile([128, 4], F32, tag="r")
        nc.vector.reciprocal(r_sb[:], pt[:, :, D : D + 1].bitcast(F32))
        o_out = osb.tile([128, 4, D], F32, tag="oo")
        nc.vector.tensor_mul(o_out[:], pt[:, :, 0:D],
                             r_sb[:].unsqueeze(2).broadcast_to([128, 4, D]))
        nc.sync.dma_start(out_ap[jg], o_out[:])
        ebusy["vec"] += 0.71

    ofifo = deque()        # (h, p, jl, eT, o_ps) awaiting O' emission
    tfifo = deque()        # pending finalize transposes, one PE op each

    def pop_oprime():
        hh, pp, jll, eTT, po = ofifo.popleft()
        oprime_j(pp, jll, eTT, po)
        if pp == NPAIR - 1:
            ot, pt = finalize_copy(hh, jll, po)
            for q in range(4):
                tfifo.append((hh, jll, ot, pt, q))

    def pop_transpose():
        hh, jll, ot, pt, q = tfifo.popleft()
        nc.tensor.transpose(pt[:, q, :], ot[:, q * 128 : (q + 1) * 128],
                            ident[0 : D + 2, 0 : D + 2])
        if q == 3:
            finalize_end(hh, jll, pt)

    for h in range(2):
        o_ps = [oacc_pool.tile([ZP, 512], F32, tag=f"o{jl}", name=f"o_ps{h}_{jl}")
                for jl in range(2)]
        for p in range(NPAIR):
            if h == 0 and p == 0:
                for jl in range(2):
                    ofifo.append((0, 0, jl, eT00, o_ps))
                continue
            if h == 0:
                z_pair_main(p)
            if h == 0 and p == 1:
                # dep-free fillers into the still-unwritten O' accumulator
                # banks (their start=True write comes slots later): PE
                # density for the HAM warmup through the main-loop ramp
                for jl in range(2):
                    nc.tensor.matmul(o_ps[jl][:, :], wtile[:, 0:ZP],
                                     wtile[:], start=True, stop=True)
            eT = new_eT(h, p)
            for jl in range(2):
                scores_exp_j(p, 2 * h + jl, jl, eT, scp)
                ofifo.append((h, p, jl, eT, o_ps))
                # steady-state lag OLAG; collapse gradually over the last
                # two pairs of the last half so the tail is short
                if h == 1 and p >= NPAIR - 2:
                    target = 2 * (NPAIR - 1 - p) + (1 - jl)
                else:
                    target = OLAG
                while len(ofifo) > target:
                    pop_oprime()
                if tfifo:
                    pop_transpose()
                    if h == 1 and p >= NPAIR - 2 and tfifo:
                        pop_transpose()

    # tail: drain the FIFOs
    while ofifo:
        pop_oprime()
        if tfifo:
            pop_transpose()
    while tfifo:
        pop_transpose()

    osb.release()
    fsb.release()
    oacc_pool.release()
    etp.release()
    scp.release()
    const.release()


_NC_CACHE = {}


def _get_nc():
    if "nc" not in _NC_CACHE:
        _NC_CACHE["nc"] = _build_nc()
    return _NC_CACHE["nc"]


def kernel(x, w_q, w_k, w_v, w_final, b_final, _trace=False):
    nc = _get_nc()
    bfr = lambda a: np.asarray(a, dtype=np.float32).astype(ml_dtypes.bfloat16).astype(np.float32)
    # weight-only folds (load-time preprocessing): A = Wq^T Wk, W2 = Wv^T Wf^T
    A = (bfr(w_q).T @ bfr(w_k)).astype(ml_dtypes.bfloat16)
    W2 = (bfr(w_v).T @ bfr(w_final).T).astype(ml_dtypes.bfloat16)
    W2d = np.ascontiguousarray(np.concatenate([W2, W2], axis=0))  # both halves
    xb = np.asarray(x, dtype=np.float32).astype(ml_dtypes.bfloat16)  # [B,S,D]
    # host-side layout marshaling: x^T per batch, duplicated onto both
    # partition halves for the row-packed score matmuls
    xT = np.ascontiguousarray(np.concatenate([xb.transpose(0, 2, 1),
                                              xb.transpose(0, 2, 1)], axis=1))
    shared = {
        "a": np.ascontiguousarray(A),
        "w2": W2d,
        "b_final": np.ascontiguousarray(np.asarray(b_final, dtype=np.float32)),
        "ident": np.eye(128, dtype=np.float32),
    }
    in_maps = [dict(shared, xT=xT[b]) for b in range(B)]
    res = run_bass_kernel_spmd(nc, in_maps, core_ids=list(range(B)), trace=_trace)
    # device output is [j, p, q, d]; un-permute to [S, 64] (layout only)
    out = np.stack([res.results[b]["out"] for b in range(B)], axis=0)
    out = np.ascontiguousarray(
        out.transpose(0, 1, 3, 2, 4).reshape(B, S, D))
    if _trace:
        return out, res
    return out
